# revision 44
# baseline (speedup 1.0000x reference)
"""GNN message-passing layer on 8 Trainium2 NeuronCores — gather + matmul-aggregation.

Per core e (one edge type per core):
    agg[t, :] = sum_{j: tgt_j = t} x[src_j, :]        (edges of type e)
    msgT_e    = W_e^T-applied transpose:  msgT[d2, t] = (agg @ W_e^T)^T
Host:  msg = sum_e msgT_e^T + sum_e outer(bincount(tgt_e), b_e); divide by counts.

Device pipeline (no scatter, no projection pass):
  - x fp16 in DRAM (host-cast). Edges target-sorted per (bucket, window),
    laid out as 4 per-bucket token streams (segments at exact harmonized
    caps, no alignment padding), sliced into fixed 1024-token gather calls.
  - dma_gather (GPSIMD/SWDGE) pulls x rows per call (~7.9 ns/token is the
    ucode floor; calls >1024 idx crash the ring).
  - Per run (segment x stripe intersection): DVE builds one-hot S [128, span]
    via is_equal (iota row vs per-partition relative target), PE matmuls
    psum_w[d, t] += Gx[stripe]^T @ S  (accumulate over runs).
  - Window retire: scalar copies psum->fp16, PE applies W_e^T, scalar
    copies fp16, sync DMA writes msgT[:, w*512:...].

GPSIMD descriptor generation is the bottleneck (~97% busy); DVE/PE/Scalar
tuck underneath. Token count = sum of per-(w,b) max-over-core counts
(harmonized schedule, shared program across cores) + per-bucket %128 tail.
"""

import numpy as np

import concourse.bacc as bacc
import concourse.bass as bass
import concourse.mybir as mybir
import concourse.tile as tile
from concourse.bass_utils import run_bass_kernel_spmd

N = 100000   # nodes
D = 128      # hidden
E = 8        # edge types == cores
M = 200000   # edges per type

NB = 4        # src buckets (int16 gather index windows)
BS = 25000    # bucket size
WD = 1024     # targets per window (2 psum banks)
NWIN = (N + WD - 1) // WD        # 196
DMA_SCRATCH = 16384              # SWDGE ring (default; larger is not faster)
MAX_CALL = 1024                  # per-gather-call index cap (ucode ring bound)
LOOKAHEAD = 16                   # windows of gather prefetch
SENT = 1200.0  # pad sentinel for relative targets (outside [0, WD))

F32 = mybir.dt.float32
F16 = mybir.dt.float16
I16 = mybir.dt.int16

TRACE = False
LAST = None


def build_schedule(edge_lists):
    """Common (cross-core) schedule + per-core index/target data.

    Token layout: 4 per-bucket streams, concatenated. Within bucket b the
    (w, b) segments sit back-to-back at exact caps[w, b] (max over cores),
    bucket tail padded to %128.  Gather calls are 1024-token slices of each
    bucket stream (last call %128).
    """
    assert edge_lists.shape == (E, M, 2)
    src = np.asarray(edge_lists[:, :, 0], dtype=np.int64)
    tgt = np.asarray(edge_lists[:, :, 1], dtype=np.int64)
    w_of = tgt // WD
    b_of = src // BS

    counts = np.zeros((E, NWIN, NB), dtype=np.int64)
    for e in range(E):
        np.add.at(counts[e], (w_of[e], b_of[e]), 1)
    caps = counts.max(axis=0)                  # [NWIN, NB], no alignment

    # per-bucket streams
    bucket_base = np.zeros(NB + 1, dtype=np.int64)
    seg_gbase = {}
    btots = []
    off = 0
    for b in range(NB):
        bucket_base[b] = off
        loc = 0
        for w in range(NWIN):
            seg_gbase[(w, b)] = off + loc
            loc += int(caps[w, b])
        loc = -(-loc // 128) * 128             # bucket tail to full stripes
        btots.append(loc)
        off += loc
    bucket_base[NB] = off
    tot = off
    assert tot % 128 == 0

    # gather calls: 1024-token slices per bucket
    calls = []            # (b, global_off, n_tokens)
    call_of_pos = {}      # bucket -> list of (start, end, call_idx)
    for b in range(NB):
        so = 0
        while so < btots[b]:
            rem = btots[b] - so
            sn = min(MAX_CALL, rem)
            if rem <= MAX_CALL:
                sn = min(256, rem)      # small tail calls unblock last windows sooner
            calls.append((b, int(bucket_base[b]) + so, sn))
            so += sn

    call_offs = np.array([c[1] for c in calls])

    def pos_to_call(gpos):
        ci = int(np.searchsorted(call_offs, gpos, side="right")) - 1
        local = gpos - calls[ci][1]
        return ci, local // 128, local % 128

    # runs: per (w, b) segment, split at 128-stripe boundaries (call
    # boundaries are %1024 so they coincide with stripe boundaries).
    runs = []
    for w in range(NWIN):
        for b in range(NB):
            cap = int(caps[w, b])
            if cap == 0:
                continue
            q = seg_gbase[(w, b)]
            rem = cap
            while rem > 0:
                ci, stripe, p0 = pos_to_call(q)
                k = min(rem, 128 - p0)
                runs.append([ci, stripe, p0, k, w, len(runs), False, False, q])
                q += k
                rem -= k

    seen_first = set()
    for r in runs:
        wv = r[4]
        if wv not in seen_first:
            r[6] = True
            seen_first.add(wv)
    nruns = len(runs)

    # per-core data
    gidx = np.zeros((E, tot), dtype=np.int16)
    trel = np.full((E, 128, nruns), SENT, dtype=np.float32)
    for e in range(E):
        order = np.lexsort((tgt[e], w_of[e], b_of[e]))
        s_srt = src[e][order]
        t_srt = tgt[e][order]
        w_srt = w_of[e][order]
        b_srt = b_of[e][order]
        key = b_srt * NWIN + w_srt
        starts = np.flatnonzero(np.r_[True, key[1:] != key[:-1]])
        ends = np.r_[starts[1:], len(key)]
        seg_sorted = {}
        for s0, s1 in zip(starts, ends):
            w = int(w_srt[s0])
            b = int(b_srt[s0])
            base = seg_gbase[(w, b)]
            gidx[e, base:base + (s1 - s0)] = (s_srt[s0:s1] % BS).astype(np.int16)
            seg_sorted[(w, b)] = (s0, s1)
        for r in runs:
            ci, stripe, p0, k, w, col, _, _, g0 = r
            b = calls[ci][0]
            ss = seg_sorted.get((w, b))
            if ss is None:
                continue
            s0, s1 = ss
            nreal = s1 - s0
            lo = g0 - seg_gbase[(w, b)]
            hi = min(lo + k, nreal)
            if hi > lo:
                rel = (t_srt[s0 + lo:s0 + hi] - w * WD).astype(np.float32)
                trel[e, p0:p0 + (hi - lo), col] = rel

    # per-run union target spans across cores
    spans = []
    trel_f32 = trel
    for r in runs:
        col = r[5]
        vals = trel_f32[:, :, col]
        real = vals < SENT
        if not real.any():
            spans.append((0, 2))
            continue
        c0 = int(vals[real].min()) & ~1
        c1 = min(WD, (int(vals[real].max()) + 2) & ~1)
        spans.append((c0, c1))

    # matmul pieces: split each run's span at the 512-col psum bank boundary;
    # mark the last piece writing each (window, bank) with stop=True.
    pieces_by_run = {}
    last_piece = {}
    for r in runs:
        col = r[5]
        c0, c1 = spans[col]
        parts = []
        a = c0
        while a < c1:
            b = min(c1, (a // 512 + 1) * 512)
            parts.append([a, b, False])
            last_piece[(r[4], a // 512)] = (col, len(parts) - 1)
            a = b
        pieces_by_run[col] = parts
    for (wv, bank), (col, pi) in last_piece.items():
        pieces_by_run[col][pi][2] = True

    gidx_w = np.tile(gidx.reshape(E, -1, 16).transpose(0, 2, 1), (1, 8, 1))

    counts_e = np.zeros((E, N), dtype=np.int64)
    for e in range(E):
        counts_e[e] = np.bincount(tgt[e], minlength=N)

    return dict(
        caps=caps, calls=calls, runs=runs, tot=tot, nruns=nruns, spans=spans,
        seg_gbase=seg_gbase, gidx=np.ascontiguousarray(gidx_w), trel=trel,
        counts_e=counts_e, pieces_by_run=pieces_by_run,
    )


def build_bass(sched):
    calls = sched["calls"]
    runs = sched["runs"]
    tot = sched["tot"]
    nruns = sched["nruns"]
    spans = sched["spans"]
    pieces = sched["pieces_by_run"]
    caps = sched["caps"]
    seg_gbase = sched["seg_gbase"]

    nc = bacc.Bacc("TRN2", target_bir_lowering=False,
                   dynamic_dma_scratch_size=DMA_SCRATCH,
                   num_swdge_queues=4)
    x_d = nc.dram_tensor("x", [N, D], F16, kind="ExternalInput")
    wt_d = nc.dram_tensor("wt", [D, D], F16, kind="ExternalInput")   # W_e^T
    gi_d = nc.dram_tensor("gidx", [128, tot // 16], I16, kind="ExternalInput")
    tr_d = nc.dram_tensor("trel", [128, nruns], F32, kind="ExternalInput")
    io_d = nc.dram_tensor("iota", [128, WD], F16, kind="ExternalInput")
    out_d = nc.dram_tensor("msgT", [128, N], F16, kind="ExternalOutput")

    runs_by_w = {}
    for r in runs:
        runs_by_w.setdefault(r[4], []).append(r)

    # per-bucket call lists and per-(window, bucket) call high-water:
    # calls of bucket b covering segment ends of windows <= w.
    call_offs = np.array([c[1] for c in calls])
    calls_of_b = {b: [ci for ci, c in enumerate(calls) if c[0] == b]
                  for b in range(NB)}
    need_b = np.zeros((NWIN, NB), dtype=np.int64)   # count within bucket list
    for b in range(NB):
        offs_b = np.array([calls[ci][1] for ci in calls_of_b[b]])
        for w in range(NWIN):
            end = seg_gbase[(w, b)] + int(caps[w, b])
            k = int(np.searchsorted(offs_b, end - 1, side="right")) if end > seg_gbase[(w, b)] else 0
            need_b[w, b] = max(k, need_b[w - 1, b] if w else 0)
        need_b[NWIN - 1, b] = len(calls_of_b[b])

    with tile.TileContext(nc) as tc:
        with (
            tc.tile_pool(name="const", bufs=1) as constp,
            tc.tile_pool(name="gx", bufs=16) as gxp,
            tc.tile_pool(name="s", bufs=10) as sp,
            tc.tile_pool(name="aggps", bufs=3, space="PSUM") as aggp,
            tc.tile_pool(name="wps", bufs=1, space="PSUM") as wpsp,
            tc.tile_pool(name="aggs", bufs=3) as aggsp,
            tc.tile_pool(name="outp", bufs=3) as outp,
        ):
            gi_s = constp.tile([128, tot // 16], I16)
            wt_s = constp.tile([D, D], F16)
            iota_s = constp.tile([128, WD], F16)
            trel_s = constp.tile([128, nruns], F32)
            # head chunks (first 2048 tokens per bucket) first: the initial
            # gather calls depend only on these tiny loads
            bstarts = sorted({min(off for (b2, off, n) in calls if b2 == b)
                              for b in range(NB)})
            bends = bstarts[1:] + [tot]
            for s in bstarts:
                nc.sync.dma_start(gi_s[:, s // 16:(s + 2048) // 16],
                                  gi_d[:, s // 16:(s + 2048) // 16])
            nc.sync.dma_start(iota_s[:], io_d[:])
            nc.sync.dma_start(trel_s[:], tr_d[:])
            nc.sync.dma_start(wt_s[:], wt_d[:])
            for s, e in zip(bstarts, bends):
                nc.sync.dma_start(gi_s[:, (s + 2048) // 16:e // 16],
                                  gi_d[:, (s + 2048) // 16:e // 16])

            # 4 calls share one tile so only 1-in-4 gathers carries a pool
            # WAR wait (head waits serialize the engine: +~500ns/call).
            gx_tiles = {}        # ci -> (tile, stripe_offset)
            qcnt = [0]           # global gather counter for queue alternation
            group_tiles = {}     # (b, k//4) -> tile
            next_b = [0] * NB

            def issue_calls(w):
                wl = min(NWIN - 1, w)
                for b in range(NB):
                    while next_b[b] < need_b[wl, b]:
                        k = next_b[b]
                        ci = calls_of_b[b][k]
                        _, off, n = calls[ci]
                        nst = -(-n // 128)
                        spc = MAX_CALL // 128
                        cpt = 32 // spc
                        gk = (b, k // cpt)
                        if gk not in group_tiles:
                            group_tiles[gk] = gxp.tile(
                                [128, 32, D], F16, tag="gx",
                                name=f"gx{gk[0]}_{gk[1]}")
                        gxt = group_tiles[gk]
                        qo = (k % cpt) * spc
                        nc.gpsimd.dma_gather(
                            gxt[:, qo:qo + nst, :], x_d[b * BS:(b + 1) * BS, :],
                            gi_s[:, off // 16:(off + n) // 16],
                            n, n, D, queue_num=qcnt[0] % 4,
                        )
                        qcnt[0] += 1
                        gx_tiles[ci] = (gxt, qo)
                        next_b[b] += 1

            retire_q = []

            def retire(w, ps):
                nwd = min(WD, N - w * WD)
                a_s = aggsp.tile([128, WD], F16, tag="aggs", name=f"aggs{w}")
                nc.scalar.copy(a_s[:], ps[:])
                wps = wpsp.tile([128, WD], F32, tag="wps", name=f"wps{w}")
                for h in range(0, WD, 512):
                    nc.tensor.matmul(wps[:, h:h + 512], wt_s[:],
                                     a_s[:, h:h + 512],
                                     start=True, stop=True,
                                     skip_group_check=True)
                o_s = outp.tile([128, WD], F16, tag="out", name=f"out{w}")
                nc.scalar.copy(o_s[:], wps[:])
                nc.sync.dma_start(out_d[:, w * WD:w * WD + nwd], o_s[:, :nwd])

            # pre-zero each window's psum one window ahead so the
            # memzero->matmul cross-engine wait is stale-satisfied
            ps_q = {}

            def prep_ps(w):
                if w < NWIN and w not in ps_q:
                    t = aggp.tile([128, WD], F32, tag="agg", name=f"agg{w}")
                    nc.scalar.memzero(t[:])
                    ps_q[w] = t

            prep_ps(0)
            for w in range(NWIN):
                issue_calls(w + LOOKAHEAD)
                prep_ps(w + 1)
                if retire_q:
                    retire(*retire_q.pop(0))
                ps = ps_q.pop(w)
                for r in runs_by_w.get(w, []):
                    ci, stripe, p0, k, _, col, first, last = r[:8]
                    gxt, qo = gx_tiles[ci]
                    c0, c1 = spans[col]
                    wc = c1 - c0
                    s_t = sp.tile([128, WD], F16, tag="s", name=f"s{col}")
                    nc.vector.tensor_scalar(
                        s_t[:, 0:wc], iota_s[:, c0:c1],
                        trel_s[:, col:col + 1], None,
                        op0=mybir.AluOpType.is_equal,
                    )
                    for (pa, pb, pstop) in pieces[col]:
                        nc.tensor.matmul(
                            ps[:, pa:pb], gxt[:, qo + stripe, :],
                            s_t[:, pa - c0:pb - c0],
                            start=False, stop=pstop, skip_group_check=True,
                        )
                retire_q.append((w, ps))
            while retire_q:
                retire(*retire_q.pop(0))

    nc.compile()
    return nc


def kernel(edge_lists, node_states, W, b):
    edge_lists = np.asarray(edge_lists)
    node_states = np.asarray(node_states, dtype=np.float32)
    W = np.asarray(W, dtype=np.float32)
    b = np.asarray(b, dtype=np.float32)

    sched = build_schedule(edge_lists)
    nc = build_bass(sched)

    x16 = node_states.astype(np.float16)
    iota = np.tile(np.arange(WD, dtype=np.float16), (128, 1))
    in_maps = []
    for e in range(E):
        wt16 = np.ascontiguousarray(W[e * D:(e + 1) * D, :].T).astype(np.float16)
        in_maps.append({
            "x": x16,
            "wt": wt16,
            "gidx": sched["gidx"][e],
            "trel": sched["trel"][e],
            "iota": iota,
        })

    global LAST
    res = run_bass_kernel_spmd(nc, in_maps, core_ids=list(range(E)), trace=TRACE)
    LAST = res

    total = np.zeros((N, D), dtype=np.float32)
    for e in range(E):
        total += res.results[e]["msgT"].astype(np.float32).T
    counts_e = sched["counts_e"].astype(np.float32)
    for e in range(E):
        total += np.outer(counts_e[e], b[e * D:(e + 1) * D])
    counts = counts_e.sum(axis=0)
    divisor = np.where(counts == 0.0, 1.0, counts)
    return (total / divisor[:, None]).astype(np.float32)


def selfcheck_schedule(edge_lists, node_states, W, b):
    """Numpy emulation of the device program for schedule validation."""
    sched = build_schedule(np.asarray(edge_lists))
    x16 = np.asarray(node_states, dtype=np.float32).astype(np.float16)
    calls, runs = sched["calls"], sched["runs"]
    total = np.zeros((N, D), dtype=np.float32)
    for e in range(E):
        gidx_w = sched["gidx"][e]
        gvals = {}
        for ci, (bkt, off, n) in enumerate(calls):
            cols = gidx_w[:16, off // 16:(off + n) // 16]
            idxs = cols.T.reshape(-1)[:n].astype(np.int64)
            rows = x16[bkt * BS + idxs]          # [n, D]
            nst = -(-n // 128)
            buf = np.zeros((128, nst, D), np.float16)
            pos = np.arange(n)
            buf[pos % 128, pos // 128] = rows
            gvals[ci] = buf
        msgT = np.zeros((128, N), dtype=np.float32)
        wt16 = np.ascontiguousarray(W[e * D:(e + 1) * D, :].T).astype(np.float16)
        trel_f32 = sched["trel"][e]
        psums = {}
        for r in runs:
            ci, stripe, p0, k, w, col, first, last = r[:8]
            if first:
                psums[w] = np.zeros((128, WD), np.float32)
            gx = gvals[ci][:, stripe, :].astype(np.float32)   # [128, D]
            rel = trel_f32[:, col]                            # [128]
            S = (rel[:, None] == np.arange(WD)[None, :]).astype(np.float32)
            psums[w] += gx.T @ S
        for w, ps in psums.items():
            nwd = min(WD, N - w * WD)
            agg16 = ps.astype(np.float16).astype(np.float32)
            m = (wt16.astype(np.float32).T @ agg16).astype(np.float16)
            msgT[:, w * WD:w * WD + nwd] = m[:, :nwd].astype(np.float32)
        total += msgT.T
    counts_e = sched["counts_e"].astype(np.float32)
    bb = np.asarray(b, dtype=np.float32)
    for e in range(E):
        total += np.outer(counts_e[e], bb[e * D:(e + 1) * D])
    counts = counts_e.sum(axis=0)
    divisor = np.where(counts == 0.0, 1.0, counts)
    return (total / divisor[:, None]).astype(np.float32)


# revision 45
# speedup vs baseline: 1.0082x; 1.0082x over previous
"""GNN message-passing layer on 8 Trainium2 NeuronCores — gather + matmul-aggregation.

Per core e (one edge type per core):
    agg[t, :] = sum_{j: tgt_j = t} x[src_j, :]        (edges of type e)
    msgT_e    = W_e^T-applied transpose:  msgT[d2, t] = (agg @ W_e^T)^T
Host:  msg = sum_e msgT_e^T + sum_e outer(bincount(tgt_e), b_e); divide by counts.

Device pipeline (no scatter, no projection pass):
  - x fp16 in DRAM (host-cast). Edges target-sorted per (bucket, window),
    laid out as 4 per-bucket token streams (segments at exact harmonized
    caps, no alignment padding), sliced into fixed 1024-token gather calls.
  - dma_gather (GPSIMD/SWDGE) pulls x rows per call (~7.9 ns/token is the
    ucode floor; calls >1024 idx crash the ring).
  - Per run (segment x stripe intersection): DVE builds one-hot S [128, span]
    via is_equal (iota row vs per-partition relative target), PE matmuls
    psum_w[d, t] += Gx[stripe]^T @ S  (accumulate over runs).
  - Window retire: scalar copies psum->fp16, PE applies W_e^T, scalar
    copies fp16, sync DMA writes msgT[:, w*512:...].

GPSIMD descriptor generation is the bottleneck (~97% busy); DVE/PE/Scalar
tuck underneath. Token count = sum of per-(w,b) max-over-core counts
(harmonized schedule, shared program across cores) + per-bucket %128 tail.
"""

import numpy as np

import concourse.bacc as bacc
import concourse.bass as bass
import concourse.mybir as mybir
import concourse.tile as tile
from concourse.bass_utils import run_bass_kernel_spmd

N = 100000   # nodes
D = 128      # hidden
E = 8        # edge types == cores
M = 200000   # edges per type

NB = 4        # src buckets (int16 gather index windows)
BS = 25000    # bucket size
WD = 1024     # targets per window (2 psum banks)
NWIN = (N + WD - 1) // WD        # 196
DMA_SCRATCH = 16384              # SWDGE ring (default; larger is not faster)
MAX_CALL = 1024                  # per-gather-call index cap (ucode ring bound)
LOOKAHEAD = 16                   # windows of gather prefetch
SENT = 1200.0  # pad sentinel for relative targets (outside [0, WD))

F32 = mybir.dt.float32
F16 = mybir.dt.float16
I16 = mybir.dt.int16

TRACE = False
LAST = None


def build_schedule(edge_lists):
    """Common (cross-core) schedule + per-core index/target data.

    Token layout: 4 per-bucket streams, concatenated. Within bucket b the
    (w, b) segments sit back-to-back at exact caps[w, b] (max over cores),
    bucket tail padded to %128.  Gather calls are 1024-token slices of each
    bucket stream (last call %128).
    """
    assert edge_lists.shape == (E, M, 2)
    src = np.asarray(edge_lists[:, :, 0], dtype=np.int64)
    tgt = np.asarray(edge_lists[:, :, 1], dtype=np.int64)
    w_of = tgt // WD
    b_of = src // BS

    counts = np.zeros((E, NWIN, NB), dtype=np.int64)
    for e in range(E):
        np.add.at(counts[e], (w_of[e], b_of[e]), 1)
    caps = counts.max(axis=0)                  # [NWIN, NB], no alignment

    # per-bucket streams
    bucket_base = np.zeros(NB + 1, dtype=np.int64)
    seg_gbase = {}
    btots = []
    off = 0
    for b in range(NB):
        bucket_base[b] = off
        loc = 0
        for w in range(NWIN):
            seg_gbase[(w, b)] = off + loc
            loc += int(caps[w, b])
        loc = -(-loc // 128) * 128             # bucket tail to full stripes
        btots.append(loc)
        off += loc
    bucket_base[NB] = off
    tot = off
    assert tot % 128 == 0

    # gather calls: 1024-token slices per bucket
    calls = []            # (b, global_off, n_tokens)
    call_of_pos = {}      # bucket -> list of (start, end, call_idx)
    for b in range(NB):
        so = 0
        while so < btots[b]:
            rem = btots[b] - so
            sn = min(MAX_CALL, rem)
            if rem <= MAX_CALL:
                sn = min(256, rem)      # small tail calls unblock last windows sooner
            calls.append((b, int(bucket_base[b]) + so, sn))
            so += sn

    call_offs = np.array([c[1] for c in calls])

    def pos_to_call(gpos):
        ci = int(np.searchsorted(call_offs, gpos, side="right")) - 1
        local = gpos - calls[ci][1]
        return ci, local // 128, local % 128

    # runs: per (w, b) segment, split at 128-stripe boundaries (call
    # boundaries are %1024 so they coincide with stripe boundaries).
    runs = []
    for w in range(NWIN):
        for b in range(NB):
            cap = int(caps[w, b])
            if cap == 0:
                continue
            q = seg_gbase[(w, b)]
            rem = cap
            while rem > 0:
                ci, stripe, p0 = pos_to_call(q)
                k = min(rem, 128 - p0)
                runs.append([ci, stripe, p0, k, w, len(runs), False, False, q])
                q += k
                rem -= k

    seen_first = set()
    for r in runs:
        wv = r[4]
        if wv not in seen_first:
            r[6] = True
            seen_first.add(wv)
    nruns = len(runs)

    # per-core data
    gidx = np.zeros((E, tot), dtype=np.int16)
    trel = np.full((E, 128, nruns), SENT, dtype=np.float32)
    for e in range(E):
        order = np.lexsort((tgt[e], w_of[e], b_of[e]))
        s_srt = src[e][order]
        t_srt = tgt[e][order]
        w_srt = w_of[e][order]
        b_srt = b_of[e][order]
        key = b_srt * NWIN + w_srt
        starts = np.flatnonzero(np.r_[True, key[1:] != key[:-1]])
        ends = np.r_[starts[1:], len(key)]
        seg_sorted = {}
        for s0, s1 in zip(starts, ends):
            w = int(w_srt[s0])
            b = int(b_srt[s0])
            base = seg_gbase[(w, b)]
            gidx[e, base:base + (s1 - s0)] = (s_srt[s0:s1] % BS).astype(np.int16)
            seg_sorted[(w, b)] = (s0, s1)
        for r in runs:
            ci, stripe, p0, k, w, col, _, _, g0 = r
            b = calls[ci][0]
            ss = seg_sorted.get((w, b))
            if ss is None:
                continue
            s0, s1 = ss
            nreal = s1 - s0
            lo = g0 - seg_gbase[(w, b)]
            hi = min(lo + k, nreal)
            if hi > lo:
                rel = (t_srt[s0 + lo:s0 + hi] - w * WD).astype(np.float32)
                trel[e, p0:p0 + (hi - lo), col] = rel

    # per-run union target spans across cores
    spans = []
    trel_f32 = trel
    for r in runs:
        col = r[5]
        vals = trel_f32[:, :, col]
        real = vals < SENT
        if not real.any():
            spans.append((0, 2))
            continue
        c0 = int(vals[real].min()) & ~1
        c1 = min(WD, (int(vals[real].max()) + 2) & ~1)
        spans.append((c0, c1))

    # matmul pieces: split each run's span at the 512-col psum bank boundary;
    # mark the last piece writing each (window, bank) with stop=True.
    pieces_by_run = {}
    last_piece = {}
    for r in runs:
        col = r[5]
        c0, c1 = spans[col]
        parts = []
        a = c0
        while a < c1:
            b = min(c1, (a // 512 + 1) * 512)
            parts.append([a, b, False])
            last_piece[(r[4], a // 512)] = (col, len(parts) - 1)
            a = b
        pieces_by_run[col] = parts
    for (wv, bank), (col, pi) in last_piece.items():
        pieces_by_run[col][pi][2] = True

    gidx_w = np.tile(gidx.reshape(E, -1, 16).transpose(0, 2, 1), (1, 8, 1))

    counts_e = np.zeros((E, N), dtype=np.int64)
    for e in range(E):
        counts_e[e] = np.bincount(tgt[e], minlength=N)

    return dict(
        caps=caps, calls=calls, runs=runs, tot=tot, nruns=nruns, spans=spans,
        seg_gbase=seg_gbase, gidx=np.ascontiguousarray(gidx_w), trel=trel,
        counts_e=counts_e, pieces_by_run=pieces_by_run,
    )


def build_bass(sched):
    calls = sched["calls"]
    runs = sched["runs"]
    tot = sched["tot"]
    nruns = sched["nruns"]
    spans = sched["spans"]
    pieces = sched["pieces_by_run"]
    caps = sched["caps"]
    seg_gbase = sched["seg_gbase"]

    nc = bacc.Bacc("TRN2", target_bir_lowering=False,
                   dynamic_dma_scratch_size=DMA_SCRATCH,
                   num_swdge_queues=4)
    x_d = nc.dram_tensor("x", [N, D], F16, kind="ExternalInput")
    wt_d = nc.dram_tensor("wt", [D, D], F16, kind="ExternalInput")   # W_e^T
    gi_d = nc.dram_tensor("gidx", [128, tot // 16], I16, kind="ExternalInput")
    tr_d = nc.dram_tensor("trel", [128, nruns], F32, kind="ExternalInput")
    io_d = nc.dram_tensor("iota", [128, WD], F16, kind="ExternalInput")
    out_d = nc.dram_tensor("msgT", [128, N], F16, kind="ExternalOutput")

    runs_by_w = {}
    for r in runs:
        runs_by_w.setdefault(r[4], []).append(r)

    # per-bucket call lists and per-(window, bucket) call high-water:
    # calls of bucket b covering segment ends of windows <= w.
    call_offs = np.array([c[1] for c in calls])
    calls_of_b = {b: [ci for ci, c in enumerate(calls) if c[0] == b]
                  for b in range(NB)}
    need_b = np.zeros((NWIN, NB), dtype=np.int64)   # count within bucket list
    for b in range(NB):
        offs_b = np.array([calls[ci][1] for ci in calls_of_b[b]])
        for w in range(NWIN):
            end = seg_gbase[(w, b)] + int(caps[w, b])
            k = int(np.searchsorted(offs_b, end - 1, side="right")) if end > seg_gbase[(w, b)] else 0
            need_b[w, b] = max(k, need_b[w - 1, b] if w else 0)
        need_b[NWIN - 1, b] = len(calls_of_b[b])

    with tile.TileContext(nc) as tc:
        with (
            tc.tile_pool(name="const", bufs=1) as constp,
            tc.tile_pool(name="gx", bufs=16) as gxp,
            tc.tile_pool(name="s", bufs=10) as sp,
            tc.tile_pool(name="aggps", bufs=3, space="PSUM") as aggp,
            tc.tile_pool(name="wps", bufs=1, space="PSUM") as wpsp,
            tc.tile_pool(name="aggs", bufs=3) as aggsp,
            tc.tile_pool(name="outp", bufs=3) as outp,
        ):
            gi_s = constp.tile([128, tot // 16], I16)
            wt_s = constp.tile([D, D], F16)
            iota_s = constp.tile([128, WD], F16)
            trel_s = constp.tile([128, nruns], F32)
            # head chunks (first 2048 tokens per bucket) first: the initial
            # gather calls depend only on these tiny loads
            bstarts = sorted({min(off for (b2, off, n) in calls if b2 == b)
                              for b in range(NB)})
            bends = bstarts[1:] + [tot]
            for s in bstarts:
                nc.sync.dma_start(gi_s[:, s // 16:(s + 2048) // 16],
                                  gi_d[:, s // 16:(s + 2048) // 16])
            nc.sync.dma_start(iota_s[:], io_d[:])
            nc.sync.dma_start(trel_s[:], tr_d[:])
            nc.sync.dma_start(wt_s[:], wt_d[:])
            for s, e in zip(bstarts, bends):
                nc.sync.dma_start(gi_s[:, (s + 2048) // 16:e // 16],
                                  gi_d[:, (s + 2048) // 16:e // 16])

            # 4 calls share one tile so only 1-in-4 gathers carries a pool
            # WAR wait (head waits serialize the engine: +~500ns/call).
            gx_tiles = {}        # ci -> (tile, stripe_offset)
            qcnt = [0]           # global gather counter for queue alternation
            group_tiles = {}     # (b, k//4) -> tile
            next_b = [0] * NB

            def issue_calls(w):
                wl = min(NWIN - 1, w)
                for b in range(NB):
                    while next_b[b] < need_b[wl, b]:
                        k = next_b[b]
                        ci = calls_of_b[b][k]
                        _, off, n = calls[ci]
                        nst = -(-n // 128)
                        spc = MAX_CALL // 128
                        cpt = 32 // spc
                        gk = (b, k // cpt)
                        if gk not in group_tiles:
                            group_tiles[gk] = gxp.tile(
                                [128, 32, D], F16, tag="gx",
                                name=f"gx{gk[0]}_{gk[1]}")
                        gxt = group_tiles[gk]
                        qo = (k % cpt) * spc
                        nc.gpsimd.dma_gather(
                            gxt[:, qo:qo + nst, :], x_d[b * BS:(b + 1) * BS, :],
                            gi_s[:, off // 16:(off + n) // 16],
                            n, n, D, queue_num=qcnt[0] % 4,
                        )
                        qcnt[0] += 1
                        gx_tiles[ci] = (gxt, qo)
                        next_b[b] += 1

            retire_q = []

            def retire(w, ps):
                nwd = min(WD, N - w * WD)
                a_s = aggsp.tile([128, WD], F16, tag="aggs", name=f"aggs{w}")
                nc.scalar.copy(a_s[:], ps[:])
                wps = wpsp.tile([128, WD], F32, tag="wps", name=f"wps{w}")
                for h in range(0, WD, 512):
                    nc.tensor.matmul(wps[:, h:h + 512], wt_s[:],
                                     a_s[:, h:h + 512],
                                     start=True, stop=True,
                                     skip_group_check=True)
                o_s = outp.tile([128, WD], F16, tag="out", name=f"out{w}")
                nc.scalar.copy(o_s[:], wps[:])
                nc.sync.dma_start(out_d[:, w * WD:w * WD + nwd], o_s[:, :nwd])

            for w in range(NWIN):
                issue_calls(w + LOOKAHEAD)
                ps = aggp.tile([128, WD], F32, tag="agg", name=f"agg{w}")
                nc.scalar.memzero(ps[:])
                for r in runs_by_w.get(w, []):
                    ci, stripe, p0, k, _, col, first, last = r[:8]
                    gxt, qo = gx_tiles[ci]
                    c0, c1 = spans[col]
                    wc = c1 - c0
                    s_t = sp.tile([128, WD], F16, tag="s", name=f"s{col}")
                    nc.vector.tensor_scalar(
                        s_t[:, 0:wc], iota_s[:, c0:c1],
                        trel_s[:, col:col + 1], None,
                        op0=mybir.AluOpType.is_equal,
                    )
                    for (pa, pb, pstop) in pieces[col]:
                        nc.tensor.matmul(
                            ps[:, pa:pb], gxt[:, qo + stripe, :],
                            s_t[:, pa - c0:pb - c0],
                            start=False, stop=pstop, skip_group_check=True,
                        )
                retire_q.append((w, ps))
                if len(retire_q) > 1:
                    retire(*retire_q.pop(0))
            while retire_q:
                retire(*retire_q.pop(0))

    nc.compile()
    return nc


def kernel(edge_lists, node_states, W, b):
    edge_lists = np.asarray(edge_lists)
    node_states = np.asarray(node_states, dtype=np.float32)
    W = np.asarray(W, dtype=np.float32)
    b = np.asarray(b, dtype=np.float32)

    sched = build_schedule(edge_lists)
    nc = build_bass(sched)

    x16 = node_states.astype(np.float16)
    iota = np.tile(np.arange(WD, dtype=np.float16), (128, 1))
    in_maps = []
    for e in range(E):
        wt16 = np.ascontiguousarray(W[e * D:(e + 1) * D, :].T).astype(np.float16)
        in_maps.append({
            "x": x16,
            "wt": wt16,
            "gidx": sched["gidx"][e],
            "trel": sched["trel"][e],
            "iota": iota,
        })

    global LAST
    res = run_bass_kernel_spmd(nc, in_maps, core_ids=list(range(E)), trace=TRACE)
    LAST = res

    total = np.zeros((N, D), dtype=np.float32)
    for e in range(E):
        total += res.results[e]["msgT"].astype(np.float32).T
    counts_e = sched["counts_e"].astype(np.float32)
    for e in range(E):
        total += np.outer(counts_e[e], b[e * D:(e + 1) * D])
    counts = counts_e.sum(axis=0)
    divisor = np.where(counts == 0.0, 1.0, counts)
    return (total / divisor[:, None]).astype(np.float32)


def selfcheck_schedule(edge_lists, node_states, W, b):
    """Numpy emulation of the device program for schedule validation."""
    sched = build_schedule(np.asarray(edge_lists))
    x16 = np.asarray(node_states, dtype=np.float32).astype(np.float16)
    calls, runs = sched["calls"], sched["runs"]
    total = np.zeros((N, D), dtype=np.float32)
    for e in range(E):
        gidx_w = sched["gidx"][e]
        gvals = {}
        for ci, (bkt, off, n) in enumerate(calls):
            cols = gidx_w[:16, off // 16:(off + n) // 16]
            idxs = cols.T.reshape(-1)[:n].astype(np.int64)
            rows = x16[bkt * BS + idxs]          # [n, D]
            nst = -(-n // 128)
            buf = np.zeros((128, nst, D), np.float16)
            pos = np.arange(n)
            buf[pos % 128, pos // 128] = rows
            gvals[ci] = buf
        msgT = np.zeros((128, N), dtype=np.float32)
        wt16 = np.ascontiguousarray(W[e * D:(e + 1) * D, :].T).astype(np.float16)
        trel_f32 = sched["trel"][e]
        psums = {}
        for r in runs:
            ci, stripe, p0, k, w, col, first, last = r[:8]
            if first:
                psums[w] = np.zeros((128, WD), np.float32)
            gx = gvals[ci][:, stripe, :].astype(np.float32)   # [128, D]
            rel = trel_f32[:, col]                            # [128]
            S = (rel[:, None] == np.arange(WD)[None, :]).astype(np.float32)
            psums[w] += gx.T @ S
        for w, ps in psums.items():
            nwd = min(WD, N - w * WD)
            agg16 = ps.astype(np.float16).astype(np.float32)
            m = (wt16.astype(np.float32).T @ agg16).astype(np.float16)
            msgT[:, w * WD:w * WD + nwd] = m[:, :nwd].astype(np.float32)
        total += msgT.T
    counts_e = sched["counts_e"].astype(np.float32)
    bb = np.asarray(b, dtype=np.float32)
    for e in range(E):
        total += np.outer(counts_e[e], bb[e * D:(e + 1) * D])
    counts = counts_e.sum(axis=0)
    divisor = np.where(counts == 0.0, 1.0, counts)
    return (total / divisor[:, None]).astype(np.float32)


# revision 46
# speedup vs baseline: 1.0102x; 1.0020x over previous
"""GNN message-passing layer on 8 Trainium2 NeuronCores — gather + matmul-aggregation.

Per core e (one edge type per core):
    agg[t, :] = sum_{j: tgt_j = t} x[src_j, :]        (edges of type e)
    msgT_e    = W_e^T-applied transpose:  msgT[d2, t] = (agg @ W_e^T)^T
Host:  msg = sum_e msgT_e^T + sum_e outer(bincount(tgt_e), b_e); divide by counts.

Device pipeline (no scatter, no projection pass):
  - x fp16 in DRAM (host-cast). Edges target-sorted per (bucket, window),
    laid out as 4 per-bucket token streams (segments at exact harmonized
    caps, no alignment padding), sliced into fixed 1024-token gather calls.
  - dma_gather (GPSIMD/SWDGE) pulls x rows per call (~7.9 ns/token is the
    ucode floor; calls >1024 idx crash the ring).
  - Per run (segment x stripe intersection): DVE builds one-hot S [128, span]
    via is_equal (iota row vs per-partition relative target), PE matmuls
    psum_w[d, t] += Gx[stripe]^T @ S  (accumulate over runs).
  - Window retire: scalar copies psum->fp16, PE applies W_e^T, scalar
    copies fp16, sync DMA writes msgT[:, w*512:...].

GPSIMD descriptor generation is the bottleneck (~97% busy); DVE/PE/Scalar
tuck underneath. Token count = sum of per-(w,b) max-over-core counts
(harmonized schedule, shared program across cores) + per-bucket %128 tail.
"""

import numpy as np

import concourse.bacc as bacc
import concourse.bass as bass
import concourse.mybir as mybir
import concourse.tile as tile
from concourse.bass_utils import run_bass_kernel_spmd

N = 100000   # nodes
D = 128      # hidden
E = 8        # edge types == cores
M = 200000   # edges per type

NB = 4        # src buckets (int16 gather index windows)
BS = 25000    # bucket size
WD = 1024     # targets per window (2 psum banks)
NWIN = (N + WD - 1) // WD        # 196
DMA_SCRATCH = 16384              # SWDGE ring (default; larger is not faster)
MAX_CALL = 1024                  # per-gather-call index cap (ucode ring bound)
LOOKAHEAD = 16                   # windows of gather prefetch
SENT = 1200.0  # pad sentinel for relative targets (outside [0, WD))

F32 = mybir.dt.float32
F16 = mybir.dt.float16
I16 = mybir.dt.int16

TRACE = False
LAST = None


def build_schedule(edge_lists):
    """Common (cross-core) schedule + per-core index/target data.

    Token layout: 4 per-bucket streams, concatenated. Within bucket b the
    (w, b) segments sit back-to-back at exact caps[w, b] (max over cores),
    bucket tail padded to %128.  Gather calls are 1024-token slices of each
    bucket stream (last call %128).
    """
    assert edge_lists.shape == (E, M, 2)
    src = np.asarray(edge_lists[:, :, 0], dtype=np.int64)
    tgt = np.asarray(edge_lists[:, :, 1], dtype=np.int64)
    w_of = tgt // WD
    b_of = src // BS

    counts = np.zeros((E, NWIN, NB), dtype=np.int64)
    for e in range(E):
        np.add.at(counts[e], (w_of[e], b_of[e]), 1)
    caps = counts.max(axis=0)                  # [NWIN, NB], no alignment

    # per-bucket streams
    bucket_base = np.zeros(NB + 1, dtype=np.int64)
    seg_gbase = {}
    btots = []
    off = 0
    for b in range(NB):
        bucket_base[b] = off
        loc = 0
        for w in range(NWIN):
            seg_gbase[(w, b)] = off + loc
            loc += int(caps[w, b])
        loc = -(-loc // 128) * 128             # bucket tail to full stripes
        btots.append(loc)
        off += loc
    bucket_base[NB] = off
    tot = off
    assert tot % 128 == 0

    # gather calls: 1024-token slices per bucket
    calls = []            # (b, global_off, n_tokens)
    call_of_pos = {}      # bucket -> list of (start, end, call_idx)
    for b in range(NB):
        so = 0
        while so < btots[b]:
            rem = btots[b] - so
            sn = min(MAX_CALL, rem)
            if rem <= MAX_CALL:
                sn = min(256, rem)      # small tail calls unblock last windows sooner
            calls.append((b, int(bucket_base[b]) + so, sn))
            so += sn

    call_offs = np.array([c[1] for c in calls])

    def pos_to_call(gpos):
        ci = int(np.searchsorted(call_offs, gpos, side="right")) - 1
        local = gpos - calls[ci][1]
        return ci, local // 128, local % 128

    # runs: per (w, b) segment, split at 128-stripe boundaries (call
    # boundaries are %1024 so they coincide with stripe boundaries).
    runs = []
    for w in range(NWIN):
        for b in range(NB):
            cap = int(caps[w, b])
            if cap == 0:
                continue
            q = seg_gbase[(w, b)]
            rem = cap
            while rem > 0:
                ci, stripe, p0 = pos_to_call(q)
                k = min(rem, 128 - p0)
                runs.append([ci, stripe, p0, k, w, len(runs), False, False, q])
                q += k
                rem -= k

    seen_first = set()
    for r in runs:
        wv = r[4]
        if wv not in seen_first:
            r[6] = True
            seen_first.add(wv)
    nruns = len(runs)

    # per-core data
    gidx = np.zeros((E, tot), dtype=np.int16)
    trel = np.full((E, 128, nruns), SENT, dtype=np.float32)
    for e in range(E):
        order = np.lexsort((tgt[e], w_of[e], b_of[e]))
        s_srt = src[e][order]
        t_srt = tgt[e][order]
        w_srt = w_of[e][order]
        b_srt = b_of[e][order]
        key = b_srt * NWIN + w_srt
        starts = np.flatnonzero(np.r_[True, key[1:] != key[:-1]])
        ends = np.r_[starts[1:], len(key)]
        seg_sorted = {}
        for s0, s1 in zip(starts, ends):
            w = int(w_srt[s0])
            b = int(b_srt[s0])
            base = seg_gbase[(w, b)]
            gidx[e, base:base + (s1 - s0)] = (s_srt[s0:s1] % BS).astype(np.int16)
            seg_sorted[(w, b)] = (s0, s1)
        for r in runs:
            ci, stripe, p0, k, w, col, _, _, g0 = r
            b = calls[ci][0]
            ss = seg_sorted.get((w, b))
            if ss is None:
                continue
            s0, s1 = ss
            nreal = s1 - s0
            lo = g0 - seg_gbase[(w, b)]
            hi = min(lo + k, nreal)
            if hi > lo:
                rel = (t_srt[s0 + lo:s0 + hi] - w * WD).astype(np.float32)
                trel[e, p0:p0 + (hi - lo), col] = rel

    # per-run union target spans across cores
    spans = []
    trel_f32 = trel
    for r in runs:
        col = r[5]
        vals = trel_f32[:, :, col]
        real = vals < SENT
        if not real.any():
            spans.append((0, 2))
            continue
        c0 = int(vals[real].min()) & ~1
        c1 = min(WD, (int(vals[real].max()) + 2) & ~1)
        spans.append((c0, c1))

    # matmul pieces: split each run's span at the 512-col psum bank boundary;
    # mark the last piece writing each (window, bank) with stop=True.
    pieces_by_run = {}
    last_piece = {}
    for r in runs:
        col = r[5]
        c0, c1 = spans[col]
        parts = []
        a = c0
        while a < c1:
            b = min(c1, (a // 512 + 1) * 512)
            parts.append([a, b, False])
            last_piece[(r[4], a // 512)] = (col, len(parts) - 1)
            a = b
        pieces_by_run[col] = parts
    for (wv, bank), (col, pi) in last_piece.items():
        pieces_by_run[col][pi][2] = True

    gidx_w = np.tile(gidx.reshape(E, -1, 16).transpose(0, 2, 1), (1, 8, 1))

    counts_e = np.zeros((E, N), dtype=np.int64)
    for e in range(E):
        counts_e[e] = np.bincount(tgt[e], minlength=N)

    return dict(
        caps=caps, calls=calls, runs=runs, tot=tot, nruns=nruns, spans=spans,
        seg_gbase=seg_gbase, gidx=np.ascontiguousarray(gidx_w), trel=trel,
        counts_e=counts_e, pieces_by_run=pieces_by_run,
    )


def build_bass(sched):
    calls = sched["calls"]
    runs = sched["runs"]
    tot = sched["tot"]
    nruns = sched["nruns"]
    spans = sched["spans"]
    pieces = sched["pieces_by_run"]
    caps = sched["caps"]
    seg_gbase = sched["seg_gbase"]

    nc = bacc.Bacc("TRN2", target_bir_lowering=False,
                   dynamic_dma_scratch_size=DMA_SCRATCH,
                   num_swdge_queues=4)
    x_d = nc.dram_tensor("x", [N, D], F16, kind="ExternalInput")
    wt_d = nc.dram_tensor("wt", [D, D], F16, kind="ExternalInput")   # W_e^T
    gi_d = nc.dram_tensor("gidx", [128, tot // 16], I16, kind="ExternalInput")
    tr_d = nc.dram_tensor("trel", [128, nruns], F32, kind="ExternalInput")
    io_d = nc.dram_tensor("iota", [128, WD], F16, kind="ExternalInput")
    out_d = nc.dram_tensor("msgT", [128, N], F16, kind="ExternalOutput")

    runs_by_w = {}
    for r in runs:
        runs_by_w.setdefault(r[4], []).append(r)

    # per-bucket call lists and per-(window, bucket) call high-water:
    # calls of bucket b covering segment ends of windows <= w.
    call_offs = np.array([c[1] for c in calls])
    calls_of_b = {b: [ci for ci, c in enumerate(calls) if c[0] == b]
                  for b in range(NB)}
    need_b = np.zeros((NWIN, NB), dtype=np.int64)   # count within bucket list
    for b in range(NB):
        offs_b = np.array([calls[ci][1] for ci in calls_of_b[b]])
        for w in range(NWIN):
            end = seg_gbase[(w, b)] + int(caps[w, b])
            k = int(np.searchsorted(offs_b, end - 1, side="right")) if end > seg_gbase[(w, b)] else 0
            need_b[w, b] = max(k, need_b[w - 1, b] if w else 0)
        need_b[NWIN - 1, b] = len(calls_of_b[b])

    with tile.TileContext(nc) as tc:
        with (
            tc.tile_pool(name="const", bufs=1) as constp,
            tc.tile_pool(name="gx", bufs=16) as gxp,
            tc.tile_pool(name="s", bufs=10) as sp,
            tc.tile_pool(name="aggps", bufs=2, space="PSUM") as aggp,
            tc.tile_pool(name="wps", bufs=2, space="PSUM") as wpsp,
            tc.tile_pool(name="aggs", bufs=3) as aggsp,
            tc.tile_pool(name="outp", bufs=3) as outp,
        ):
            gi_s = constp.tile([128, tot // 16], I16)
            wt_s = constp.tile([D, D], F16)
            iota_s = constp.tile([128, WD], F16)
            trel_s = constp.tile([128, nruns], F32)
            # head chunks (first 2048 tokens per bucket) first: the initial
            # gather calls depend only on these tiny loads
            bstarts = sorted({min(off for (b2, off, n) in calls if b2 == b)
                              for b in range(NB)})
            bends = bstarts[1:] + [tot]
            for s in bstarts:
                nc.sync.dma_start(gi_s[:, s // 16:(s + 2048) // 16],
                                  gi_d[:, s // 16:(s + 2048) // 16])
            nc.sync.dma_start(iota_s[:], io_d[:])
            nc.sync.dma_start(trel_s[:], tr_d[:])
            nc.sync.dma_start(wt_s[:], wt_d[:])
            for s, e in zip(bstarts, bends):
                nc.sync.dma_start(gi_s[:, (s + 2048) // 16:e // 16],
                                  gi_d[:, (s + 2048) // 16:e // 16])

            # 4 calls share one tile so only 1-in-4 gathers carries a pool
            # WAR wait (head waits serialize the engine: +~500ns/call).
            gx_tiles = {}        # ci -> (tile, stripe_offset)
            qcnt = [0]           # global gather counter for queue alternation
            group_tiles = {}     # (b, k//4) -> tile
            next_b = [0] * NB

            def issue_calls(w):
                wl = min(NWIN - 1, w)
                for b in range(NB):
                    while next_b[b] < need_b[wl, b]:
                        k = next_b[b]
                        ci = calls_of_b[b][k]
                        _, off, n = calls[ci]
                        nst = -(-n // 128)
                        spc = MAX_CALL // 128
                        cpt = 32 // spc
                        gk = (b, k // cpt)
                        if gk not in group_tiles:
                            group_tiles[gk] = gxp.tile(
                                [128, 32, D], F16, tag="gx",
                                name=f"gx{gk[0]}_{gk[1]}")
                        gxt = group_tiles[gk]
                        qo = (k % cpt) * spc
                        nc.gpsimd.dma_gather(
                            gxt[:, qo:qo + nst, :], x_d[b * BS:(b + 1) * BS, :],
                            gi_s[:, off // 16:(off + n) // 16],
                            n, n, D, queue_num=qcnt[0] % 4,
                        )
                        qcnt[0] += 1
                        gx_tiles[ci] = (gxt, qo)
                        next_b[b] += 1

            retire_q = []

            def retire(w, ps):
                nwd = min(WD, N - w * WD)
                a_s = aggsp.tile([128, WD], F16, tag="aggs", name=f"aggs{w}")
                nc.scalar.copy(a_s[:], ps[:])
                wps = wpsp.tile([128, WD], F32, tag="wps", name=f"wps{w}")
                for h in range(0, WD, 512):
                    nc.tensor.matmul(wps[:, h:h + 512], wt_s[:],
                                     a_s[:, h:h + 512],
                                     start=True, stop=True,
                                     skip_group_check=True)
                o_s = outp.tile([128, WD], F16, tag="out", name=f"out{w}")
                nc.scalar.copy(o_s[:], wps[:])
                nc.sync.dma_start(out_d[:, w * WD:w * WD + nwd], o_s[:, :nwd])

            for w in range(NWIN):
                issue_calls(w + LOOKAHEAD)
                ps = aggp.tile([128, WD], F32, tag="agg", name=f"agg{w}")
                nc.scalar.memzero(ps[:])
                for r in runs_by_w.get(w, []):
                    ci, stripe, p0, k, _, col, first, last = r[:8]
                    gxt, qo = gx_tiles[ci]
                    c0, c1 = spans[col]
                    wc = c1 - c0
                    s_t = sp.tile([128, WD], F16, tag="s", name=f"s{col}")
                    nc.vector.tensor_scalar(
                        s_t[:, 0:wc], iota_s[:, c0:c1],
                        trel_s[:, col:col + 1], None,
                        op0=mybir.AluOpType.is_equal,
                    )
                    for (pa, pb, pstop) in pieces[col]:
                        nc.tensor.matmul(
                            ps[:, pa:pb], gxt[:, qo + stripe, :],
                            s_t[:, pa - c0:pb - c0],
                            start=False, stop=pstop, skip_group_check=True,
                        )
                retire_q.append((w, ps))
                if len(retire_q) > 1:
                    retire(*retire_q.pop(0))
            while retire_q:
                retire(*retire_q.pop(0))

    nc.compile()
    return nc


def kernel(edge_lists, node_states, W, b):
    edge_lists = np.asarray(edge_lists)
    node_states = np.asarray(node_states, dtype=np.float32)
    W = np.asarray(W, dtype=np.float32)
    b = np.asarray(b, dtype=np.float32)

    sched = build_schedule(edge_lists)
    nc = build_bass(sched)

    x16 = node_states.astype(np.float16)
    iota = np.tile(np.arange(WD, dtype=np.float16), (128, 1))
    in_maps = []
    for e in range(E):
        wt16 = np.ascontiguousarray(W[e * D:(e + 1) * D, :].T).astype(np.float16)
        in_maps.append({
            "x": x16,
            "wt": wt16,
            "gidx": sched["gidx"][e],
            "trel": sched["trel"][e],
            "iota": iota,
        })

    global LAST
    res = run_bass_kernel_spmd(nc, in_maps, core_ids=list(range(E)), trace=TRACE)
    LAST = res

    total = np.zeros((N, D), dtype=np.float32)
    for e in range(E):
        total += res.results[e]["msgT"].astype(np.float32).T
    counts_e = sched["counts_e"].astype(np.float32)
    for e in range(E):
        total += np.outer(counts_e[e], b[e * D:(e + 1) * D])
    counts = counts_e.sum(axis=0)
    divisor = np.where(counts == 0.0, 1.0, counts)
    return (total / divisor[:, None]).astype(np.float32)


def selfcheck_schedule(edge_lists, node_states, W, b):
    """Numpy emulation of the device program for schedule validation."""
    sched = build_schedule(np.asarray(edge_lists))
    x16 = np.asarray(node_states, dtype=np.float32).astype(np.float16)
    calls, runs = sched["calls"], sched["runs"]
    total = np.zeros((N, D), dtype=np.float32)
    for e in range(E):
        gidx_w = sched["gidx"][e]
        gvals = {}
        for ci, (bkt, off, n) in enumerate(calls):
            cols = gidx_w[:16, off // 16:(off + n) // 16]
            idxs = cols.T.reshape(-1)[:n].astype(np.int64)
            rows = x16[bkt * BS + idxs]          # [n, D]
            nst = -(-n // 128)
            buf = np.zeros((128, nst, D), np.float16)
            pos = np.arange(n)
            buf[pos % 128, pos // 128] = rows
            gvals[ci] = buf
        msgT = np.zeros((128, N), dtype=np.float32)
        wt16 = np.ascontiguousarray(W[e * D:(e + 1) * D, :].T).astype(np.float16)
        trel_f32 = sched["trel"][e]
        psums = {}
        for r in runs:
            ci, stripe, p0, k, w, col, first, last = r[:8]
            if first:
                psums[w] = np.zeros((128, WD), np.float32)
            gx = gvals[ci][:, stripe, :].astype(np.float32)   # [128, D]
            rel = trel_f32[:, col]                            # [128]
            S = (rel[:, None] == np.arange(WD)[None, :]).astype(np.float32)
            psums[w] += gx.T @ S
        for w, ps in psums.items():
            nwd = min(WD, N - w * WD)
            agg16 = ps.astype(np.float16).astype(np.float32)
            m = (wt16.astype(np.float32).T @ agg16).astype(np.float16)
            msgT[:, w * WD:w * WD + nwd] = m[:, :nwd].astype(np.float32)
        total += msgT.T
    counts_e = sched["counts_e"].astype(np.float32)
    bb = np.asarray(b, dtype=np.float32)
    for e in range(E):
        total += np.outer(counts_e[e], bb[e * D:(e + 1) * D])
    counts = counts_e.sum(axis=0)
    divisor = np.where(counts == 0.0, 1.0, counts)
    return (total / divisor[:, None]).astype(np.float32)


# revision 47
# speedup vs baseline: 1.0262x; 1.0159x over previous
"""GNN message-passing layer on 8 Trainium2 NeuronCores — gather + matmul-aggregation.

Per core e (one edge type per core):
    agg[t, :] = sum_{j: tgt_j = t} x[src_j, :]        (edges of type e)
    msgT_e    = W_e^T-applied transpose:  msgT[d2, t] = (agg @ W_e^T)^T
Host:  msg = sum_e msgT_e^T + sum_e outer(bincount(tgt_e), b_e); divide by counts.

Device pipeline (no scatter, no projection pass):
  - x fp16 in DRAM (host-cast). Edges target-sorted per (bucket, window),
    laid out as 4 per-bucket token streams (segments at exact harmonized
    caps, no alignment padding), sliced into fixed 1024-token gather calls.
  - dma_gather (GPSIMD/SWDGE) pulls x rows per call (~7.9 ns/token is the
    ucode floor; calls >1024 idx crash the ring).
  - Per run (segment x stripe intersection): DVE builds one-hot S [128, span]
    via is_equal (iota row vs per-partition relative target), PE matmuls
    psum_w[d, t] += Gx[stripe]^T @ S  (accumulate over runs).
  - Window retire: scalar copies psum->fp16, PE applies W_e^T, scalar
    copies fp16, sync DMA writes msgT[:, w*512:...].

GPSIMD descriptor generation is the bottleneck (~97% busy); DVE/PE/Scalar
tuck underneath. Token count = sum of per-(w,b) max-over-core counts
(harmonized schedule, shared program across cores) + per-bucket %128 tail.
"""

import numpy as np

import concourse.bacc as bacc
import concourse.bass as bass
import concourse.mybir as mybir
import concourse.tile as tile
from concourse.bass_utils import run_bass_kernel_spmd

N = 100000   # nodes
D = 128      # hidden
E = 8        # edge types == cores
M = 200000   # edges per type

NB = 4        # src buckets (int16 gather index windows)
BS = 25000    # bucket size
WD = 1024     # targets per window (2 psum banks)
NWIN = (N + WD - 1) // WD        # 196
DMA_SCRATCH = 16384              # SWDGE ring (default; larger is not faster)
MAX_CALL = 1024                  # per-gather-call index cap (ucode ring bound)
LOOKAHEAD = 16                   # windows of gather prefetch
SENT = 1200.0  # pad sentinel for relative targets (outside [0, WD))

F32 = mybir.dt.float32
F16 = mybir.dt.float16
I16 = mybir.dt.int16

TRACE = False
LAST = None


def build_schedule(edge_lists):
    """Common (cross-core) schedule + per-core index/target data.

    Token layout: 4 per-bucket streams, concatenated. Within bucket b the
    (w, b) segments sit back-to-back at exact caps[w, b] (max over cores),
    bucket tail padded to %128.  Gather calls are 1024-token slices of each
    bucket stream (last call %128).
    """
    assert edge_lists.shape == (E, M, 2)
    src = np.asarray(edge_lists[:, :, 0], dtype=np.int64)
    tgt = np.asarray(edge_lists[:, :, 1], dtype=np.int64)
    w_of = tgt // WD
    b_of = src // BS

    counts = np.zeros((E, NWIN, NB), dtype=np.int64)
    for e in range(E):
        np.add.at(counts[e], (w_of[e], b_of[e]), 1)
    caps = counts.max(axis=0)                  # [NWIN, NB], no alignment

    # per-bucket streams
    bucket_base = np.zeros(NB + 1, dtype=np.int64)
    seg_gbase = {}
    btots = []
    off = 0
    for b in range(NB):
        bucket_base[b] = off
        loc = 0
        for w in range(NWIN):
            seg_gbase[(w, b)] = off + loc
            loc += int(caps[w, b])
        loc = -(-loc // 128) * 128             # bucket tail to full stripes
        btots.append(loc)
        off += loc
    bucket_base[NB] = off
    tot = off
    assert tot % 128 == 0

    # gather calls: 1024-token slices per bucket
    calls = []            # (b, global_off, n_tokens)
    call_of_pos = {}      # bucket -> list of (start, end, call_idx)
    for b in range(NB):
        so = 0
        while so < btots[b]:
            rem = btots[b] - so
            sn = min(MAX_CALL, rem)
            if rem <= MAX_CALL:
                sn = min(256, rem)      # small tail calls unblock last windows sooner
            calls.append((b, int(bucket_base[b]) + so, sn))
            so += sn

    call_offs = np.array([c[1] for c in calls])

    def pos_to_call(gpos):
        ci = int(np.searchsorted(call_offs, gpos, side="right")) - 1
        local = gpos - calls[ci][1]
        return ci, local // 128, local % 128

    # runs: per (w, b) segment, split at 128-stripe boundaries (call
    # boundaries are %1024 so they coincide with stripe boundaries).
    runs = []
    for w in range(NWIN):
        for b in range(NB):
            cap = int(caps[w, b])
            if cap == 0:
                continue
            q = seg_gbase[(w, b)]
            rem = cap
            while rem > 0:
                ci, stripe, p0 = pos_to_call(q)
                k = min(rem, 128 - p0)
                runs.append([ci, stripe, p0, k, w, len(runs), False, False, q])
                q += k
                rem -= k

    seen_first = set()
    for r in runs:
        wv = r[4]
        if wv not in seen_first:
            r[6] = True
            seen_first.add(wv)
    nruns = len(runs)

    # per-core data
    gidx = np.zeros((E, tot), dtype=np.int16)
    trel = np.full((E, 128, nruns), SENT, dtype=np.float32)
    for e in range(E):
        order = np.lexsort((tgt[e], w_of[e], b_of[e]))
        s_srt = src[e][order]
        t_srt = tgt[e][order]
        w_srt = w_of[e][order]
        b_srt = b_of[e][order]
        key = b_srt * NWIN + w_srt
        starts = np.flatnonzero(np.r_[True, key[1:] != key[:-1]])
        ends = np.r_[starts[1:], len(key)]
        seg_sorted = {}
        for s0, s1 in zip(starts, ends):
            w = int(w_srt[s0])
            b = int(b_srt[s0])
            base = seg_gbase[(w, b)]
            gidx[e, base:base + (s1 - s0)] = (s_srt[s0:s1] % BS).astype(np.int16)
            seg_sorted[(w, b)] = (s0, s1)
        for r in runs:
            ci, stripe, p0, k, w, col, _, _, g0 = r
            b = calls[ci][0]
            ss = seg_sorted.get((w, b))
            if ss is None:
                continue
            s0, s1 = ss
            nreal = s1 - s0
            lo = g0 - seg_gbase[(w, b)]
            hi = min(lo + k, nreal)
            if hi > lo:
                rel = (t_srt[s0 + lo:s0 + hi] - w * WD).astype(np.float32)
                trel[e, p0:p0 + (hi - lo), col] = rel

    # per-run union target spans across cores
    spans = []
    trel_f32 = trel
    for r in runs:
        col = r[5]
        vals = trel_f32[:, :, col]
        real = vals < SENT
        if not real.any():
            spans.append((0, 2))
            continue
        c0 = int(vals[real].min()) & ~1
        c1 = min(WD, (int(vals[real].max()) + 2) & ~1)
        spans.append((c0, c1))

    # matmul pieces: split each run's span at the 512-col psum bank boundary;
    # mark the last piece writing each (window, bank) with stop=True.
    pieces_by_run = {}
    last_piece = {}
    for r in runs:
        col = r[5]
        c0, c1 = spans[col]
        parts = []
        a = c0
        while a < c1:
            b = min(c1, (a // 512 + 1) * 512)
            parts.append([a, b, False])
            last_piece[(r[4], a // 512)] = (col, len(parts) - 1)
            a = b
        pieces_by_run[col] = parts
    for (wv, bank), (col, pi) in last_piece.items():
        pieces_by_run[col][pi][2] = True

    gidx_w = np.tile(gidx.reshape(E, -1, 16).transpose(0, 2, 1), (1, 8, 1))

    counts_e = np.zeros((E, N), dtype=np.int64)
    for e in range(E):
        counts_e[e] = np.bincount(tgt[e], minlength=N)

    return dict(
        caps=caps, calls=calls, runs=runs, tot=tot, nruns=nruns, spans=spans,
        seg_gbase=seg_gbase, gidx=np.ascontiguousarray(gidx_w), trel=trel,
        counts_e=counts_e, pieces_by_run=pieces_by_run,
    )


def build_bass(sched):
    calls = sched["calls"]
    runs = sched["runs"]
    tot = sched["tot"]
    nruns = sched["nruns"]
    spans = sched["spans"]
    pieces = sched["pieces_by_run"]
    caps = sched["caps"]
    seg_gbase = sched["seg_gbase"]

    nc = bacc.Bacc("TRN2", target_bir_lowering=False,
                   dynamic_dma_scratch_size=DMA_SCRATCH,
                   num_swdge_queues=4)
    x_d = nc.dram_tensor("x", [N, D], F16, kind="ExternalInput")
    wt_d = nc.dram_tensor("wt", [D, D], F16, kind="ExternalInput")   # W_e^T
    gi_d = nc.dram_tensor("gidx", [128, tot // 16], I16, kind="ExternalInput")
    tr_d = nc.dram_tensor("trel", [128, nruns], F32, kind="ExternalInput")
    io_d = nc.dram_tensor("iota", [128, WD], F16, kind="ExternalInput")
    out_d = nc.dram_tensor("msgT", [128, N], F16, kind="ExternalOutput")

    runs_by_w = {}
    for r in runs:
        runs_by_w.setdefault(r[4], []).append(r)

    # per-bucket call lists and per-(window, bucket) call high-water:
    # calls of bucket b covering segment ends of windows <= w.
    call_offs = np.array([c[1] for c in calls])
    calls_of_b = {b: [ci for ci, c in enumerate(calls) if c[0] == b]
                  for b in range(NB)}
    need_b = np.zeros((NWIN, NB), dtype=np.int64)   # count within bucket list
    for b in range(NB):
        offs_b = np.array([calls[ci][1] for ci in calls_of_b[b]])
        for w in range(NWIN):
            end = seg_gbase[(w, b)] + int(caps[w, b])
            k = int(np.searchsorted(offs_b, end - 1, side="right")) if end > seg_gbase[(w, b)] else 0
            need_b[w, b] = max(k, need_b[w - 1, b] if w else 0)
        need_b[NWIN - 1, b] = len(calls_of_b[b])

    with tile.TileContext(nc) as tc:
        with (
            tc.tile_pool(name="const", bufs=1) as constp,
            tc.tile_pool(name="gx", bufs=16) as gxp,
            tc.tile_pool(name="s", bufs=10) as sp,
            tc.tile_pool(name="aggps", bufs=3, space="PSUM") as aggp,
            tc.tile_pool(name="wps", bufs=1, space="PSUM") as wpsp,
            tc.tile_pool(name="aggs", bufs=3) as aggsp,
            tc.tile_pool(name="outp", bufs=3) as outp,
        ):
            gi_s = constp.tile([128, tot // 16], I16)
            wt_s = constp.tile([D, D], F16)
            iota_s = constp.tile([128, WD], F16)
            trel_s = constp.tile([128, nruns], F32)
            # head chunks (first 2048 tokens per bucket) first: the initial
            # gather calls depend only on these tiny loads
            bstarts = sorted({min(off for (b2, off, n) in calls if b2 == b)
                              for b in range(NB)})
            bends = bstarts[1:] + [tot]
            for s in bstarts:
                nc.sync.dma_start(gi_s[:, s // 16:(s + 2048) // 16],
                                  gi_d[:, s // 16:(s + 2048) // 16])
            nc.sync.dma_start(iota_s[:], io_d[:])
            nc.sync.dma_start(trel_s[:], tr_d[:])
            nc.sync.dma_start(wt_s[:], wt_d[:])
            for s, e in zip(bstarts, bends):
                nc.sync.dma_start(gi_s[:, (s + 2048) // 16:e // 16],
                                  gi_d[:, (s + 2048) // 16:e // 16])

            # 4 calls share one tile so only 1-in-4 gathers carries a pool
            # WAR wait (head waits serialize the engine: +~500ns/call).
            gx_tiles = {}        # ci -> (tile, stripe_offset)
            qcnt = [0]           # global gather counter for queue alternation
            group_tiles = {}     # (b, k//4) -> tile
            next_b = [0] * NB

            def issue_calls(w):
                wl = min(NWIN - 1, w)
                for b in range(NB):
                    while next_b[b] < need_b[wl, b]:
                        k = next_b[b]
                        ci = calls_of_b[b][k]
                        _, off, n = calls[ci]
                        nst = -(-n // 128)
                        spc = MAX_CALL // 128
                        cpt = 32 // spc
                        gk = (b, k // cpt)
                        if gk not in group_tiles:
                            group_tiles[gk] = gxp.tile(
                                [128, 32, D], F16, tag="gx",
                                name=f"gx{gk[0]}_{gk[1]}")
                        gxt = group_tiles[gk]
                        qo = (k % cpt) * spc
                        nc.gpsimd.dma_gather(
                            gxt[:, qo:qo + nst, :], x_d[b * BS:(b + 1) * BS, :],
                            gi_s[:, off // 16:(off + n) // 16],
                            n, n, D, queue_num=qcnt[0] % 4,
                        )
                        qcnt[0] += 1
                        gx_tiles[ci] = (gxt, qo)
                        next_b[b] += 1

            retire_q = []

            def retire(w, ps):
                nwd = min(WD, N - w * WD)
                a_s = aggsp.tile([128, WD], F16, tag="aggs", name=f"aggs{w}")
                nc.scalar.copy(a_s[:], ps[:])
                wps = wpsp.tile([128, WD], F32, tag="wps", name=f"wps{w}")
                for h in range(0, WD, 512):
                    nc.tensor.matmul(wps[:, h:h + 512], wt_s[:],
                                     a_s[:, h:h + 512],
                                     start=True, stop=True,
                                     skip_group_check=True)
                o_s = outp.tile([128, WD], F16, tag="out", name=f"out{w}")
                nc.scalar.copy(o_s[:], wps[:])
                nc.sync.dma_start(out_d[:, w * WD:w * WD + nwd], o_s[:, :nwd])

            for w in range(NWIN):
                issue_calls(w + LOOKAHEAD)
                ps = aggp.tile([128, WD], F32, tag="agg", name=f"agg{w}")
                nc.scalar.memzero(ps[:])
                for r in runs_by_w.get(w, []):
                    ci, stripe, p0, k, _, col, first, last = r[:8]
                    gxt, qo = gx_tiles[ci]
                    c0, c1 = spans[col]
                    wc = c1 - c0
                    s_t = sp.tile([128, WD], F16, tag="s", name=f"s{col}")
                    nc.vector.tensor_scalar(
                        s_t[:, 0:wc], iota_s[:, c0:c1],
                        trel_s[:, col:col + 1], None,
                        op0=mybir.AluOpType.is_equal,
                    )
                    for (pa, pb, pstop) in pieces[col]:
                        nc.tensor.matmul(
                            ps[:, pa:pb], gxt[:, qo + stripe, :],
                            s_t[:, pa - c0:pb - c0],
                            start=False, stop=pstop, skip_group_check=True,
                        )
                retire_q.append((w, ps))
                if len(retire_q) > 1:
                    retire(*retire_q.pop(0))
            while retire_q:
                retire(*retire_q.pop(0))

    nc.compile()
    return nc


def kernel(edge_lists, node_states, W, b):
    edge_lists = np.asarray(edge_lists)
    node_states = np.asarray(node_states, dtype=np.float32)
    W = np.asarray(W, dtype=np.float32)
    b = np.asarray(b, dtype=np.float32)

    sched = build_schedule(edge_lists)
    nc = build_bass(sched)

    x16 = node_states.astype(np.float16)
    iota = np.tile(np.arange(WD, dtype=np.float16), (128, 1))
    in_maps = []
    for e in range(E):
        wt16 = np.ascontiguousarray(W[e * D:(e + 1) * D, :].T).astype(np.float16)
        in_maps.append({
            "x": x16,
            "wt": wt16,
            "gidx": sched["gidx"][e],
            "trel": sched["trel"][e],
            "iota": iota,
        })

    global LAST
    res = run_bass_kernel_spmd(nc, in_maps, core_ids=list(range(E)), trace=TRACE)
    LAST = res

    total = np.zeros((N, D), dtype=np.float32)
    for e in range(E):
        total += res.results[e]["msgT"].astype(np.float32).T
    counts_e = sched["counts_e"].astype(np.float32)
    for e in range(E):
        total += np.outer(counts_e[e], b[e * D:(e + 1) * D])
    counts = counts_e.sum(axis=0)
    divisor = np.where(counts == 0.0, 1.0, counts)
    return (total / divisor[:, None]).astype(np.float32)


def selfcheck_schedule(edge_lists, node_states, W, b):
    """Numpy emulation of the device program for schedule validation."""
    sched = build_schedule(np.asarray(edge_lists))
    x16 = np.asarray(node_states, dtype=np.float32).astype(np.float16)
    calls, runs = sched["calls"], sched["runs"]
    total = np.zeros((N, D), dtype=np.float32)
    for e in range(E):
        gidx_w = sched["gidx"][e]
        gvals = {}
        for ci, (bkt, off, n) in enumerate(calls):
            cols = gidx_w[:16, off // 16:(off + n) // 16]
            idxs = cols.T.reshape(-1)[:n].astype(np.int64)
            rows = x16[bkt * BS + idxs]          # [n, D]
            nst = -(-n // 128)
            buf = np.zeros((128, nst, D), np.float16)
            pos = np.arange(n)
            buf[pos % 128, pos // 128] = rows
            gvals[ci] = buf
        msgT = np.zeros((128, N), dtype=np.float32)
        wt16 = np.ascontiguousarray(W[e * D:(e + 1) * D, :].T).astype(np.float16)
        trel_f32 = sched["trel"][e]
        psums = {}
        for r in runs:
            ci, stripe, p0, k, w, col, first, last = r[:8]
            if first:
                psums[w] = np.zeros((128, WD), np.float32)
            gx = gvals[ci][:, stripe, :].astype(np.float32)   # [128, D]
            rel = trel_f32[:, col]                            # [128]
            S = (rel[:, None] == np.arange(WD)[None, :]).astype(np.float32)
            psums[w] += gx.T @ S
        for w, ps in psums.items():
            nwd = min(WD, N - w * WD)
            agg16 = ps.astype(np.float16).astype(np.float32)
            m = (wt16.astype(np.float32).T @ agg16).astype(np.float16)
            msgT[:, w * WD:w * WD + nwd] = m[:, :nwd].astype(np.float32)
        total += msgT.T
    counts_e = sched["counts_e"].astype(np.float32)
    bb = np.asarray(b, dtype=np.float32)
    for e in range(E):
        total += np.outer(counts_e[e], bb[e * D:(e + 1) * D])
    counts = counts_e.sum(axis=0)
    divisor = np.where(counts == 0.0, 1.0, counts)
    return (total / divisor[:, None]).astype(np.float32)
